# revision 29
# baseline (speedup 1.0000x reference)
"""ChunkedDiagonalMLP Trainium2 kernel — 8-core SPMD, data-parallel over tokens.

Math (per token row x of width 4096, split into 8 chunks of 512):
    h_n  = gelu(x_n @ w1[n] + b1[n])          (exact erf gelu)
    y_n  = h_n @ w2[n] + b2[n]
    out  = LayerNorm(concat_n(y_n) + x) * ln_g + ln_b

Per core (2048 tokens), software-pipelined over 16 chunk-PAIR blocks
(4 token groups x 4 pairs), L2 lagging L1 by one pair so the PE never
waits on the gelu handoff:
  - both layers fp8e4 DoubleRow (x, w1, h, w2 quantized; scales undone
    downstream); PE work = 2 matmuls per 256-wide contraction
  - layer 1 feature-major: stationary w1, moving x^T (host-pre-transposed
    fp8) -> h^T in PSUM; gelu reads a 2-bank [128,2,512] PSUM tile in ONE
    ScalarE instruction (b1==0 fast path) -> fp8 h^T
  - layer 2 token-major: chunk pairs share a 2-bank [128,1024] PSUM tile;
    ONE DVE scalar_tensor_tensor per pair does y/W2_SCALE + x (residual,
    bf16 token-major x) with accum_out -> per-pair token sums
  - LN stats: per 128-token tile, one batched sum-of-squares pass over
    [128,4096] split by columns DVE stt / ScalarE Square+accum_out to
    balance engines; sqrt batched per group (one act-table swap);
    apply on DVE tensor_scalar (4x mode on bf16)
  - group G's LN tail (squares/stats/sqrt/apply/store) is DEFERRED and
    spread one piece per block into groups G+1/G+2 so no engine queue
    sees a burst that would starve the PE of gelu/psum handoffs
  - DMA queues: SP ring = xr stream, Act ring = weights (JIT) + xT,
    stores alternate SWDGE (Pool) / Act so input prefetch never queues
    behind stores
"""

import numpy as np
import ml_dtypes
from contextlib import ExitStack

import concourse.bass as bass
import concourse.mybir as mybir
import concourse.tile as tile
from concourse.bass_utils import run_bass_kernel_spmd

N_CORES = 8
D = 4096
NCH = 8          # chunks
NP = NCH // 2    # chunk pairs (4)
CH = 512         # chunk width
KT = CH // 128   # k-tiles per chunk (4)
S = 2048         # tokens per core
TG = 512         # tokens per group
NG = S // TG     # 4 groups
GIT = TG // 128  # 128-token tiles per group (4)
EPS = 1e-5

F32 = mybir.dt.float32
BF16 = mybir.dt.bfloat16
F8 = mybir.dt.float8e4
BF = ml_dtypes.bfloat16
NPF8 = ml_dtypes.float8_e4m3
W2_SCALE = 256.0  # keeps fp8 w2 out of the subnormal range; undone in the stt
FP8 = True       # fp8e4 DoubleRow for layer 2 (bf16 fallback when False)
FP8_L1 = True    # fp8e4 DoubleRow for layer 1 (x and w1 quantized on host)
W1_SCALE = 256.0  # keeps fp8 w1 out of the subnormal range; undone in the gelu
DR = mybir.MatmulPerfMode.DoubleRow


def _split_excess_waits(nc, limit=1):
    """walrus CoreV3 codegen rejects instructions with too many sem waits
    (Drain allows only 1); move extras onto preceding same-engine NoOps."""
    n_split = 0
    for bb in nc.main_func.blocks:
        new_insts = []
        changed = False
        for inst in bb.instructions:
            lim = limit
            si = inst.sync_info
            if si is not None and si.on_wait and len(si.on_wait) > lim:
                waits = list(si.on_wait)
                extra, keep = waits[:-lim], waits[-lim:]
                for i in range(0, len(extra), lim):
                    nop = mybir.InstNoOp(
                        name=f"{inst.name}-ws{i}",
                        engine=inst.engine,
                        ins=[],
                        outs=[],
                        sync_info=mybir.SyncInfo(
                            on_wait=list(extra[i : i + lim]), on_update=[]
                        ),
                    )
                    new_insts.append(nop)
                    n_split += 1
                inst.sync_info = mybir.SyncInfo(
                    on_wait=list(keep), on_update=list(si.on_update)
                )
                changed = True
            new_insts.append(inst)
        if changed:
            bb.instructions[:] = new_insts
    return n_split


def _build(use_b2, use_lng, use_lnb, reps=1, x_bufs=4, h_bufs=3, ph_bufs=2,
           py_bufs=2, o_bufs=2, pf=2, sq_dve_cols=1536, use_b1=False,
           store_rings="gs"):
    groups = [(gi * TG, TG) for gi in range(NG)]
    nc = bass.Bass()
    # x^T per core: [n, k, c(128), t] fp8 (layer-1 input only; the residual
    # path reads the separate bf16 token-major copy)
    xT_e = nc.declare_dram_parameter(
        "xT", [NCH, KT, 128, S], F8 if FP8_L1 else BF16, isOutput=False
    )
    # token-major x rows (bf16) for residual
    xr_e = nc.declare_dram_parameter("xr", [S, D], BF16, isOutput=False)
    # weights: [n, c(128), k, d] (host pre-permuted so partition lines
    # are contiguous)
    w1_e = nc.declare_dram_parameter(
        "w1", [NCH, 128, KT, CH], F8 if FP8_L1 else BF16, isOutput=False
    )
    w2_e = nc.declare_dram_parameter(
        "w2", [NCH, 128, KT, CH], F8 if FP8 else BF16, isOutput=False
    )
    # b1 rearranged to [128, n*4+j] columns
    b1_e = nc.declare_dram_parameter("b1c", [128, NCH * KT], F32, isOutput=False)
    b2_e = nc.declare_dram_parameter("b2", [NCH, CH], F32, isOutput=False)
    lng_e = nc.declare_dram_parameter("ln_g", [D], F32, isOutput=False)
    lnb_e = nc.declare_dram_parameter("ln_b", [D], F32, isOutput=False)
    out_e = nc.declare_dram_parameter("out", [S, D], BF16, isOutput=True)

    with tile.TileContext(nc) as tc:
        with ExitStack() as ctx:
            opool = ctx.enter_context(tc.tile_pool(name="opool", bufs=o_bufs))
            xpool = ctx.enter_context(tc.tile_pool(name="xpool", bufs=x_bufs))
            hpool = ctx.enter_context(tc.tile_pool(name="hpool", bufs=h_bufs))
            spool = ctx.enter_context(tc.tile_pool(name="spool", bufs=2))
            cpool = ctx.enter_context(tc.tile_pool(name="cpool", bufs=1))
            pp_h = ctx.enter_context(tc.tile_pool(name="pp_h", bufs=ph_bufs, space="PSUM"))
            pp_y = ctx.enter_context(tc.tile_pool(name="pp_y", bufs=py_bufs, space="PSUM"))

            # ---- constants / weights (resident), all on the Act HWDGE ring
            # so the SP ring starts the first xT loads immediately ----
            b1_sb = cpool.tile([128, NCH * KT], F32)
            nc.scalar.dma_start(out=b1_sb, in_=b1_e[:, :])
            eps_sb = cpool.tile([128, 1], F32)
            nc.vector.memset(eps_sb, EPS)

            # Weight tiles are declared up front but DMA'd just-in-time from
            # inside the first group's pair loop so the start of the kernel
            # isn't DMA-bound on the weights.
            w1_sb = [
                cpool.tile([128, KT, CH], F8 if FP8_L1 else BF16, name=f"w1_{n}")
                for n in range(NCH)
            ]
            w2_sb = [
                cpool.tile([128, KT, CH], F8 if FP8 else BF16, name=f"w2_{n}")
                for n in range(NCH)
            ]
            w_loaded = set()

            def load_w(which, n):
                if n < NCH and (which, n) not in w_loaded:
                    w_loaded.add((which, n))
                    t, e = (w1_sb, w1_e) if which == 1 else (w2_sb, w2_e)
                    nc.scalar.dma_start(out=t[n], in_=e[n])

            load_w(1, 0)
            load_w(1, 1)
            load_w(2, 0)
            load_w(2, 1)
            load_w(1, 2)
            load_w(1, 3)

            b2_sb = None
            if use_b2:
                b2_sb = cpool.tile([128, NCH, CH], F32)
                nc.gpsimd.dma_start(
                    out=b2_sb,
                    in_=bass.AP(
                        tensor=b2_e.tensor,
                        offset=b2_e.offset,
                        ap=[[0, 128], b2_e.ap[0], b2_e.ap[1]],
                    ),
                )
            lng_sb = None
            if use_lng:
                lng_sb = cpool.tile([128, D], F32)
                nc.gpsimd.dma_start(
                    out=lng_sb,
                    in_=bass.AP(
                        tensor=lng_e.tensor, offset=lng_e.offset,
                        ap=[[0, 128], lng_e.ap[0]],
                    ),
                )
            lnb_sb = None
            if use_lnb:
                lnb_sb = cpool.tile([128, D], F32)
                nc.gpsimd.dma_start(
                    out=lnb_sb,
                    in_=bass.AP(
                        tensor=lnb_e.tensor, offset=lnb_e.offset,
                        ap=[[0, 128], lnb_e.ap[0]],
                    ),
                )

            # flat schedule across reps: block b = (rep, group, pair); LN
            # stats/apply for group G are DEFERRED into group G+1/G+2's
            # blocks so no engine sees a burst at group boundaries
            blocks = [(g % NG, p) for g in range(NG * reps) for p in range(NP)]
            nblocks = len(blocks)
            xT_tiles = {}
            xr_tiles = {}
            hT_tiles = {}
            gstate = {}

            if True:
                def emit_xdma(idx):
                    g, p = blocks[idx]
                    n0 = 2 * p
                    gpos, gt = groups[g]
                    tsl = slice(gpos, gpos + gt)
                    xT_sb = xpool.tile(
                        [128, 2, KT, gt], F8 if FP8_L1 else BF16, name="xT_sb",
                        bufs=pf + 2,
                    )
                    # Act-ring DMA: xr alone on the SP ring; real HWDGE
                    # triggers are cheap for the issuing engine
                    nc.scalar.dma_start(
                        out=xT_sb,
                        in_=xT_e[n0 : n0 + 2, :, :, tsl].rearrange(
                            "n k c t -> c n k t"
                        ),
                    )
                    xT_tiles[idx] = xT_sb
                    xr_sb = xpool.tile(
                        [128, GIT, 2 * CH], BF16, name="xr_sb", bufs=pf + 2
                    )
                    nc.sync.dma_start(
                        out=xr_sb,
                        in_=xr_e[tsl, n0 * CH : (n0 + 2) * CH].rearrange(
                            "(i p) d -> p i d", p=128
                        ),
                    )
                    xr_tiles[idx] = xr_sb

                def emit_L1(idx):
                    g, p = blocks[idx]
                    gt = groups[g][1]
                    xT_sb = xT_tiles.pop(idx)
                    hT = hpool.tile(
                        [128, 2, KT, gt], F8 if FP8 else BF16, name="hT"
                    )
                    hT_tiles[idx] = hT
                    for nn in range(2):
                        n = 2 * p + nn
                        for jp in range(KT // 2):
                            ph = pp_h.tile([128, 2, gt], F32, tag="ph", name="ph")
                            for jj in range(2):
                                j = 2 * jp + jj
                                if FP8_L1:
                                    for a in range(KT // 2):
                                        nc.tensor.matmul(
                                            ph[:, jj, :],
                                            w1_sb[n][:, 2 * a : 2 * a + 2,
                                                     j * 128 : (j + 1) * 128],
                                            xT_sb[:, nn, 2 * a : 2 * a + 2, :],
                                            start=(a == 0),
                                            stop=(a == KT // 2 - 1),
                                            perf_mode=DR,
                                        )
                                else:
                                    for k in range(KT):
                                        nc.tensor.matmul(
                                            ph[:, jj, :],
                                            w1_sb[n][:, k, j * 128 : (j + 1) * 128],
                                            xT_sb[:, nn, k, :],
                                            start=(k == 0),
                                            stop=(k == KT - 1),
                                        )
                            if use_b1:
                                # general path: per-j bias columns
                                for jj in range(2):
                                    j = 2 * jp + jj
                                    nc.scalar.activation(
                                        out=hT[:, nn, j, :],
                                        in_=ph[:, jj, :],
                                        func=mybir.ActivationFunctionType.Gelu,
                                        bias=b1_sb[:, n * KT + j : n * KT + j + 1],
                                        scale=(1.0 / W1_SCALE) if FP8_L1 else 1.0,
                                    )
                            else:
                                # b1 == 0: one gelu over the 2-bank PSUM tile
                                nc.scalar.activation(
                                    out=hT[:, nn, 2 * jp : 2 * jp + 2, :],
                                    in_=ph,
                                    func=mybir.ActivationFunctionType.Gelu,
                                    bias=0.0,
                                    scale=(1.0 / W1_SCALE) if FP8_L1 else 1.0,
                                )

                def emit_sq_stats(fg, i):
                    """after outs[i] complete: sum-of-squares pass split
                    DVE/ScalarE by columns, then mu/var smalls."""
                    sums, sqsa, sqs, mus, gvar, outs = gstate[fg]
                    sqb = spool.tile([128, D], BF16, tag="sqb", name="sqb", bufs=2)
                    dc = sq_dve_cols
                    if dc > 0:
                        nc.vector.scalar_tensor_tensor(
                            out=sqb[:, :dc],
                            in0=outs[i][:, :dc],
                            scalar=1.0,
                            in1=outs[i][:, :dc],
                            op0=mybir.AluOpType.mult,
                            op1=mybir.AluOpType.mult,
                            accum_out=sqsa[:, i : i + 1],
                        )
                    if dc < D:
                        nc.scalar.activation(
                            out=sqb[:, dc:],
                            in_=outs[i][:, dc:],
                            func=mybir.ActivationFunctionType.Square,
                            accum_out=sqs[:, i : i + 1],
                        )
                    with tc.high_priority():
                        # mu = sum(sums_pairs)/D
                        nc.vector.tensor_reduce(
                            out=mus[:, i : i + 1], in_=sums[:, i, :],
                            axis=mybir.AxisListType.X, op=mybir.AluOpType.add,
                        )
                        nc.vector.tensor_scalar_mul(
                            out=mus[:, i : i + 1], in0=mus[:, i : i + 1],
                            scalar1=1.0 / D,
                        )
                        # var = (sqsa+sqs)/D - mu^2
                        if dc > 0 and dc < D:
                            nc.vector.tensor_add(
                                out=sqs[:, i : i + 1], in0=sqs[:, i : i + 1],
                                in1=sqsa[:, i : i + 1],
                            )
                        elif dc == D:
                            nc.vector.tensor_copy(
                                out=sqs[:, i : i + 1], in_=sqsa[:, i : i + 1]
                            )
                        mu2 = spool.tile([128, 1], F32, name="mu2", bufs=4)
                        nc.vector.tensor_mul(
                            out=mu2, in0=mus[:, i : i + 1], in1=mus[:, i : i + 1]
                        )
                        nc.vector.scalar_tensor_tensor(
                            out=gvar[:, i : i + 1],
                            in0=sqs[:, i : i + 1],
                            scalar=1.0 / D,
                            in1=mu2,
                            op0=mybir.AluOpType.mult,
                            op1=mybir.AluOpType.subtract,
                        )

                def emit_sqrt(fg):
                    """one sqrt for the whole group (single act-table swap)"""
                    sums, sqsa, sqs, mus, gvar, outs = gstate[fg]
                    git = len(outs)
                    srt = spool.tile([128, GIT], F32, name="srt", bufs=2)
                    gstate[fg] = (sums, sqsa, sqs, mus, gvar, outs, srt)
                    with tc.high_priority():
                        nc.scalar.activation(
                            out=srt[:, :git], in_=gvar[:, :git],
                            func=mybir.ActivationFunctionType.Sqrt,
                            bias=eps_sb,
                        )

                def emit_apply_store(fg, which):
                    sums, sqsa, sqs, mus, gvar, outs, srt = gstate[fg]
                    g = fg % NG
                    gpos, gt = groups[g]
                    tsl = slice(gpos, gpos + gt)
                    for i in which:
                        with tc.high_priority():
                            rs = spool.tile([128, 1], F32, name="rs", bufs=4)
                            nc.vector.reciprocal(out=rs, in_=srt[:, i : i + 1])
                        nc.vector.tensor_scalar(
                            out=outs[i],
                            in0=outs[i],
                            scalar1=mus[:, i : i + 1],
                            scalar2=rs,
                            op0=mybir.AluOpType.subtract,
                            op1=mybir.AluOpType.mult,
                        )
                        if use_lng:
                            nc.vector.tensor_mul(out=outs[i], in0=outs[i], in1=lng_sb)
                        if use_lnb:
                            nc.vector.tensor_add(out=outs[i], in0=outs[i], in1=lnb_sb)
                        store_eng = {"g": nc.gpsimd, "s": nc.scalar, "y": nc.sync}[
                            store_rings[i % len(store_rings)]
                        ]
                        store_eng.dma_start(
                            out=out_e[tsl, :].rearrange("(i p) d -> p i d", p=128)[
                                :, i, :
                            ],
                            in_=outs[i],
                        )

                def emit_L2(idx):
                    g, p = blocks[idx]
                    fg = idx // NP
                    n0 = 2 * p
                    gt = groups[g][1]
                    git = gt // 128
                    if p == 0:
                        sums = spool.tile([128, GIT, NP], F32, name="sums")
                        sqsa = spool.tile([128, GIT], F32, name="sqsa")
                        sqs = spool.tile([128, GIT], F32, name="sqs")
                        mus = spool.tile([128, GIT], F32, name="mus")
                        gvar = spool.tile([128, GIT], F32, name="gvar")
                        outs = [
                            opool.tile([128, D], BF16, name=f"o{i}")
                            for i in range(git)
                        ]
                        gstate[fg] = (sums, sqsa, sqs, mus, gvar, outs)
                    sums, sqsa, sqs, mus, gvar, outs = gstate[fg][:6]
                    hT = hT_tiles.pop(idx)
                    xr_sb = xr_tiles.pop(idx)
                    for i in range(git):
                        py = pp_y.tile([128, 2 * CH], F32, tag="py", name="py")
                        for nn in range(2):
                            n = n0 + nn
                            if FP8:
                                for a in range(KT // 2):
                                    nc.tensor.matmul(
                                        py[:, nn * CH : (nn + 1) * CH],
                                        hT[:, nn, 2 * a : 2 * a + 2,
                                           i * 128 : (i + 1) * 128],
                                        w2_sb[n][:, 2 * a : 2 * a + 2, :],
                                        start=(a == 0),
                                        stop=(a == KT // 2 - 1),
                                        perf_mode=DR,
                                    )
                            else:
                                for j in range(KT):
                                    nc.tensor.matmul(
                                        py[:, nn * CH : (nn + 1) * CH],
                                        hT[:, nn, j, i * 128 : (i + 1) * 128],
                                        w2_sb[n][:, j, :],
                                        start=(j == 0),
                                        stop=(j == KT - 1),
                                    )
                        osl = outs[i][:, n0 * CH : (n0 + 2) * CH]
                        # out = y/W2_SCALE + x ; accumulate per-token pair sums
                        nc.vector.scalar_tensor_tensor(
                            out=osl,
                            in0=py,
                            scalar=(1.0 / W2_SCALE) if FP8 else 1.0,
                            in1=xr_sb[:, i, :],
                            op0=mybir.AluOpType.mult,
                            op1=mybir.AluOpType.add,
                            accum_out=sums[:, i, p : p + 1],
                        )
                        if use_b2:
                            for nn in range(2):
                                nc.vector.tensor_add(
                                    outs[i][:, (n0 + nn) * CH : (n0 + nn + 1) * CH],
                                    outs[i][:, (n0 + nn) * CH : (n0 + nn + 1) * CH],
                                    b2_sb[:, n0 + nn, :],
                                )

                def emit_deferred(idx):
                    """LN tail work for earlier groups, spread one piece per
                    block: squares+stats for group fg-1 (tile p); at p==3
                    sqrt + applies {0,1}; at p==0 applies {2,3} of fg-2."""
                    fg, p = idx // NP, idx % NP
                    if p == 0 and fg >= 2 and (fg - 2) in gstate:
                        emit_apply_store(fg - 2, range(2, len(gstate[fg - 2][5])))
                        gstate.pop(fg - 2)
                    if fg >= 1 and (fg - 1) in gstate:
                        git = len(gstate[fg - 1][5])
                        if p < git:
                            emit_sq_stats(fg - 1, p)
                        if p == NP - 1:
                            emit_sqrt(fg - 1)
                            emit_apply_store(fg - 1, range(0, 2))

                for idx in range(min(pf, nblocks)):
                    emit_xdma(idx)
                for idx in range(nblocks):
                    g, p = blocks[idx]
                    if idx < NP:
                        n0 = 2 * p
                        load_w(1, n0 + 4)
                        load_w(1, n0 + 5)
                        load_w(2, n0 + 2)
                        load_w(2, n0 + 3)
                    if idx + pf < nblocks:
                        emit_xdma(idx + pf)
                    emit_L1(idx)
                    if idx >= 1:
                        emit_L2(idx - 1)
                    emit_deferred(idx)
                emit_L2(nblocks - 1)
                # epilogue: flush deferred LN tail for the final groups
                for idx in range(nblocks, nblocks + 2 * NP):
                    emit_deferred(idx)
                assert not gstate, (list(gstate),)

    _split_excess_waits(nc)
    return nc


_CACHE = {}


def prep_inputs(inputs):
    """Host-side sharding + layout prep -> per-core input maps."""
    x = np.asarray(inputs["x"])
    w1 = np.asarray(inputs["w1"], dtype=np.float32)
    w2 = np.asarray(inputs["w2"], dtype=np.float32)
    b1 = np.asarray(inputs["b1"], dtype=np.float32)
    b2 = np.asarray(inputs["b2"], dtype=np.float32)
    ln_g = np.asarray(inputs["ln_g"], dtype=np.float32)
    ln_b = np.asarray(inputs["ln_b"], dtype=np.float32)
    B, L, d = x.shape
    x2 = np.ascontiguousarray(x.reshape(B * L, D).astype(np.float32))
    _w1p = w1.reshape(NCH, KT, 128, CH).transpose(0, 2, 1, 3)
    w1h = np.ascontiguousarray(
        (_w1p * W1_SCALE).astype(NPF8) if FP8_L1 else _w1p.astype(BF)
    )
    _w2p = w2.reshape(NCH, KT, 128, CH).transpose(0, 2, 1, 3)
    w2h = np.ascontiguousarray(
        (_w2p * W2_SCALE).astype(NPF8) if FP8 else _w2p.astype(BF)
    )
    b1h = np.ascontiguousarray(
        b1.reshape(NCH, KT, 128).transpose(2, 0, 1).reshape(128, NCH * KT)
    )

    in_maps = []
    for c in range(N_CORES):
        rows = x2[c * S : (c + 1) * S]  # [S, D] fp32
        xTh = (
            np.ascontiguousarray(rows.T)
            .astype(NPF8 if FP8_L1 else BF)
            .reshape(NCH, KT, 128, S)
        )
        in_maps.append(
            {
                "xT": xTh,
                "xr": rows.astype(BF),
                "w1": w1h,
                "w2": w2h,
                "b1c": b1h,
                "b2": b2,
                "ln_g": ln_g,
                "ln_b": ln_b,
            }
        )
    return in_maps


def kernel(x, w1, b1, w2, b2, ln_g, ln_b):
    x = np.asarray(x)
    b1 = np.asarray(b1, dtype=np.float32)
    b2 = np.asarray(b2, dtype=np.float32)
    ln_g = np.asarray(ln_g, dtype=np.float32)
    ln_b = np.asarray(ln_b, dtype=np.float32)
    B, L, d = x.shape
    assert d == D and B * L == N_CORES * S, (x.shape,)

    use_b1 = bool(np.any(b1 != 0.0))
    use_b2 = bool(np.any(b2 != 0.0))
    use_lng = bool(np.any(ln_g != 1.0))
    use_lnb = bool(np.any(ln_b != 0.0))

    key = (use_b2, use_lng, use_lnb, use_b1)
    if key not in _CACHE:
        _CACHE[key] = _build(*key[:3], use_b1=use_b1)
    nc = _CACHE[key]

    in_maps = prep_inputs(
        {"x": x, "w1": w1, "b1": b1, "w2": w2, "b2": b2, "ln_g": ln_g, "ln_b": ln_b}
    )

    res = run_bass_kernel_spmd(nc, in_maps, list(range(N_CORES)))
    out = np.concatenate([res.results[c]["out"] for c in range(N_CORES)], axis=0)
    return out.reshape(B, L, D).astype(np.float32)


# revision 30
# speedup vs baseline: 1.4076x; 1.4076x over previous
"""ChunkedDiagonalMLP Trainium2 kernel — 8-core SPMD, data-parallel over tokens.

Math (per token row x of width 4096, split into 8 chunks of 512):
    h_n  = gelu(x_n @ w1[n] + b1[n])          (exact erf gelu)
    y_n  = h_n @ w2[n] + b2[n]
    out  = LayerNorm(concat_n(y_n) + x) * ln_g + ln_b

Per core (2048 tokens), software-pipelined over 16 chunk-PAIR blocks
(4 token groups x 4 pairs), L2 lagging L1 by one pair so the PE never
waits on the gelu handoff:
  - both layers fp8e4 DoubleRow (x, w1, h, w2 quantized; scales undone
    downstream); PE work = 2 matmuls per 256-wide contraction
  - layer 1 feature-major: stationary w1, moving x^T (host-pre-transposed
    fp8) -> h^T in PSUM; gelu reads a 2-bank [128,2,512] PSUM tile in ONE
    ScalarE instruction (b1==0 fast path) -> fp8 h^T
  - layer 2 token-major: chunk pairs share a 2-bank [128,1024] PSUM tile;
    ONE DVE scalar_tensor_tensor per pair does y/W2_SCALE + x (residual,
    bf16 token-major x) with accum_out -> per-pair token sums
  - LN stats: per 128-token tile, one batched sum-of-squares pass over
    [128,4096] split by columns DVE stt / ScalarE Square+accum_out to
    balance engines; sqrt batched per group (one act-table swap);
    apply on DVE tensor_scalar (4x mode on bf16)
  - group G's LN tail (squares/stats/sqrt/apply/store) is DEFERRED and
    spread one piece per block into groups G+1/G+2 so no engine queue
    sees a burst that would starve the PE of gelu/psum handoffs
  - DMA queues: SP ring = xr stream, Act ring = weights (JIT) + xT,
    stores alternate SWDGE (Pool) / Act so input prefetch never queues
    behind stores
"""

import numpy as np
import ml_dtypes
from contextlib import ExitStack

import concourse.bass as bass
import concourse.mybir as mybir
import concourse.tile as tile
from concourse.bass_utils import run_bass_kernel_spmd

N_CORES = 8
D = 4096
NCH = 8          # chunks
NP = NCH // 2    # chunk pairs (4)
CH = 512         # chunk width
KT = CH // 128   # k-tiles per chunk (4)
S = 2048         # tokens per core
TG = 512         # tokens per group
NG = S // TG     # 4 groups
GIT = TG // 128  # 128-token tiles per group (4)
EPS = 1e-5

F32 = mybir.dt.float32
BF16 = mybir.dt.bfloat16
F8 = mybir.dt.float8e4
BF = ml_dtypes.bfloat16
NPF8 = ml_dtypes.float8_e4m3
W2_SCALE = 256.0  # keeps fp8 w2 out of the subnormal range; undone in the stt
FP8 = True       # fp8e4 DoubleRow for layer 2 (bf16 fallback when False)
FP8_L1 = True    # fp8e4 DoubleRow for layer 1 (x and w1 quantized on host)
W1_SCALE = 256.0  # keeps fp8 w1 out of the subnormal range; undone in the gelu
DR = mybir.MatmulPerfMode.DoubleRow


def _split_excess_waits(nc, limit=1):
    """walrus CoreV3 codegen rejects instructions with too many sem waits
    (Drain allows only 1); move extras onto preceding same-engine NoOps."""
    n_split = 0
    for bb in nc.main_func.blocks:
        new_insts = []
        changed = False
        for inst in bb.instructions:
            lim = limit
            si = inst.sync_info
            if si is not None and si.on_wait and len(si.on_wait) > lim:
                waits = list(si.on_wait)
                extra, keep = waits[:-lim], waits[-lim:]
                for i in range(0, len(extra), lim):
                    nop = mybir.InstNoOp(
                        name=f"{inst.name}-ws{i}",
                        engine=inst.engine,
                        ins=[],
                        outs=[],
                        sync_info=mybir.SyncInfo(
                            on_wait=list(extra[i : i + lim]), on_update=[]
                        ),
                    )
                    new_insts.append(nop)
                    n_split += 1
                inst.sync_info = mybir.SyncInfo(
                    on_wait=list(keep), on_update=list(si.on_update)
                )
                changed = True
            new_insts.append(inst)
        if changed:
            bb.instructions[:] = new_insts
    return n_split


def _build(use_b2, use_lng, use_lnb, reps=1, x_bufs=4, h_bufs=4, ph_bufs=2,
           py_bufs=2, o_bufs=2, pf=2, sq_dve_cols=1280, use_b1=False,
           store_rings="gs"):
    groups = [(gi * TG, TG) for gi in range(NG)]
    nc = bass.Bass()
    # x^T per core: [n, k, c(128), t] fp8 (layer-1 input only; the residual
    # path reads the separate bf16 token-major copy)
    xT_e = nc.declare_dram_parameter(
        "xT", [NCH, KT, 128, S], F8 if FP8_L1 else BF16, isOutput=False
    )
    # token-major x rows (bf16) for residual
    xr_e = nc.declare_dram_parameter("xr", [S, D], BF16, isOutput=False)
    # weights: [n, c(128), k, d] (host pre-permuted so partition lines
    # are contiguous)
    w1_e = nc.declare_dram_parameter(
        "w1", [NCH, 128, KT, CH], F8 if FP8_L1 else BF16, isOutput=False
    )
    w2_e = nc.declare_dram_parameter(
        "w2", [NCH, 128, KT, CH], F8 if FP8 else BF16, isOutput=False
    )
    # b1 rearranged to [128, n*4+j] columns
    b1_e = nc.declare_dram_parameter("b1c", [128, NCH * KT], F32, isOutput=False)
    b2_e = nc.declare_dram_parameter("b2", [NCH, CH], F32, isOutput=False)
    lng_e = nc.declare_dram_parameter("ln_g", [D], F32, isOutput=False)
    lnb_e = nc.declare_dram_parameter("ln_b", [D], F32, isOutput=False)
    out_e = nc.declare_dram_parameter("out", [S, D], BF16, isOutput=True)

    with tile.TileContext(nc) as tc:
        with ExitStack() as ctx:
            opool = ctx.enter_context(tc.tile_pool(name="opool", bufs=o_bufs))
            xpool = ctx.enter_context(tc.tile_pool(name="xpool", bufs=x_bufs))
            hpool = ctx.enter_context(tc.tile_pool(name="hpool", bufs=h_bufs))
            spool = ctx.enter_context(tc.tile_pool(name="spool", bufs=2))
            cpool = ctx.enter_context(tc.tile_pool(name="cpool", bufs=1))
            pp_h = ctx.enter_context(tc.tile_pool(name="pp_h", bufs=ph_bufs, space="PSUM"))
            pp_y = ctx.enter_context(tc.tile_pool(name="pp_y", bufs=py_bufs, space="PSUM"))

            # ---- constants / weights (resident), all on the Act HWDGE ring
            # so the SP ring starts the first xT loads immediately ----
            b1_sb = cpool.tile([128, NCH * KT], F32)
            nc.scalar.dma_start(out=b1_sb, in_=b1_e[:, :])
            eps_sb = cpool.tile([128, 1], F32)
            nc.vector.memset(eps_sb, EPS)

            # Weight tiles are declared up front but DMA'd just-in-time from
            # inside the first group's pair loop so the start of the kernel
            # isn't DMA-bound on the weights.
            w1_sb = [
                cpool.tile([128, KT, CH], F8 if FP8_L1 else BF16, name=f"w1_{n}")
                for n in range(NCH)
            ]
            w2_sb = [
                cpool.tile([128, KT, CH], F8 if FP8 else BF16, name=f"w2_{n}")
                for n in range(NCH)
            ]
            w_loaded = set()

            def load_w(which, n):
                if n < NCH and (which, n) not in w_loaded:
                    w_loaded.add((which, n))
                    t, e = (w1_sb, w1_e) if which == 1 else (w2_sb, w2_e)
                    nc.scalar.dma_start(out=t[n], in_=e[n])

            load_w(1, 0)
            load_w(1, 1)
            load_w(2, 0)
            load_w(2, 1)
            load_w(1, 2)
            load_w(1, 3)

            b2_sb = None
            if use_b2:
                b2_sb = cpool.tile([128, NCH, CH], F32)
                nc.gpsimd.dma_start(
                    out=b2_sb,
                    in_=bass.AP(
                        tensor=b2_e.tensor,
                        offset=b2_e.offset,
                        ap=[[0, 128], b2_e.ap[0], b2_e.ap[1]],
                    ),
                )
            lng_sb = None
            if use_lng:
                lng_sb = cpool.tile([128, D], F32)
                nc.gpsimd.dma_start(
                    out=lng_sb,
                    in_=bass.AP(
                        tensor=lng_e.tensor, offset=lng_e.offset,
                        ap=[[0, 128], lng_e.ap[0]],
                    ),
                )
            lnb_sb = None
            if use_lnb:
                lnb_sb = cpool.tile([128, D], F32)
                nc.gpsimd.dma_start(
                    out=lnb_sb,
                    in_=bass.AP(
                        tensor=lnb_e.tensor, offset=lnb_e.offset,
                        ap=[[0, 128], lnb_e.ap[0]],
                    ),
                )

            # flat schedule across reps: block b = (rep, group, pair); LN
            # stats/apply for group G are DEFERRED into group G+1/G+2's
            # blocks so no engine sees a burst at group boundaries
            blocks = [(g % NG, p) for g in range(NG * reps) for p in range(NP)]
            nblocks = len(blocks)
            xT_tiles = {}
            xr_tiles = {}
            hT_tiles = {}
            gstate = {}

            if True:
                def emit_xdma(idx):
                    g, p = blocks[idx]
                    n0 = 2 * p
                    gpos, gt = groups[g]
                    tsl = slice(gpos, gpos + gt)
                    xT_sb = xpool.tile(
                        [128, 2, KT, gt], F8 if FP8_L1 else BF16, name="xT_sb",
                        bufs=pf + 2,
                    )
                    # Act-ring DMA: xr alone on the SP ring; real HWDGE
                    # triggers are cheap for the issuing engine
                    nc.scalar.dma_start(
                        out=xT_sb,
                        in_=xT_e[n0 : n0 + 2, :, :, tsl].rearrange(
                            "n k c t -> c n k t"
                        ),
                    )
                    xT_tiles[idx] = xT_sb
                    xr_sb = xpool.tile(
                        [128, GIT, 2 * CH], BF16, name="xr_sb", bufs=pf + 2
                    )
                    nc.sync.dma_start(
                        out=xr_sb,
                        in_=xr_e[tsl, n0 * CH : (n0 + 2) * CH].rearrange(
                            "(i p) d -> p i d", p=128
                        ),
                    )
                    xr_tiles[idx] = xr_sb

                def emit_L1(idx):
                    g, p = blocks[idx]
                    gt = groups[g][1]
                    xT_sb = xT_tiles.pop(idx)
                    hT = hpool.tile(
                        [128, 2, KT, gt], F8 if FP8 else BF16, name="hT"
                    )
                    hT_tiles[idx] = hT
                    for nn in range(2):
                        n = 2 * p + nn
                        for jp in range(KT // 2):
                            ph = pp_h.tile([128, 2, gt], F32, tag="ph", name="ph")
                            for jj in range(2):
                                j = 2 * jp + jj
                                if FP8_L1:
                                    for a in range(KT // 2):
                                        nc.tensor.matmul(
                                            ph[:, jj, :],
                                            w1_sb[n][:, 2 * a : 2 * a + 2,
                                                     j * 128 : (j + 1) * 128],
                                            xT_sb[:, nn, 2 * a : 2 * a + 2, :],
                                            start=(a == 0),
                                            stop=(a == KT // 2 - 1),
                                            perf_mode=DR,
                                        )
                                else:
                                    for k in range(KT):
                                        nc.tensor.matmul(
                                            ph[:, jj, :],
                                            w1_sb[n][:, k, j * 128 : (j + 1) * 128],
                                            xT_sb[:, nn, k, :],
                                            start=(k == 0),
                                            stop=(k == KT - 1),
                                        )
                            if use_b1:
                                # general path: per-j bias columns
                                for jj in range(2):
                                    j = 2 * jp + jj
                                    nc.scalar.activation(
                                        out=hT[:, nn, j, :],
                                        in_=ph[:, jj, :],
                                        func=mybir.ActivationFunctionType.Gelu,
                                        bias=b1_sb[:, n * KT + j : n * KT + j + 1],
                                        scale=(1.0 / W1_SCALE) if FP8_L1 else 1.0,
                                    )
                            else:
                                # b1 == 0: one gelu over the 2-bank PSUM tile
                                nc.scalar.activation(
                                    out=hT[:, nn, 2 * jp : 2 * jp + 2, :],
                                    in_=ph,
                                    func=mybir.ActivationFunctionType.Gelu,
                                    bias=0.0,
                                    scale=(1.0 / W1_SCALE) if FP8_L1 else 1.0,
                                )

                def emit_sq_stats(fg, i):
                    """after outs[i] complete: sum-of-squares pass split
                    DVE/ScalarE by columns, then mu/var smalls."""
                    sums, sqsa, sqs, mus, gvar, outs = gstate[fg]
                    sqb = spool.tile([128, D], BF16, tag="sqb", name="sqb", bufs=2)
                    dc = sq_dve_cols
                    if dc > 0:
                        nc.vector.scalar_tensor_tensor(
                            out=sqb[:, :dc],
                            in0=outs[i][:, :dc],
                            scalar=1.0,
                            in1=outs[i][:, :dc],
                            op0=mybir.AluOpType.mult,
                            op1=mybir.AluOpType.mult,
                            accum_out=sqsa[:, i : i + 1],
                        )
                    if dc < D:
                        nc.scalar.activation(
                            out=sqb[:, dc:],
                            in_=outs[i][:, dc:],
                            func=mybir.ActivationFunctionType.Square,
                            accum_out=sqs[:, i : i + 1],
                        )
                    with tc.high_priority():
                        # mu = sum(sums_pairs)/D
                        nc.vector.tensor_reduce(
                            out=mus[:, i : i + 1], in_=sums[:, i, :],
                            axis=mybir.AxisListType.X, op=mybir.AluOpType.add,
                        )
                        nc.vector.tensor_scalar_mul(
                            out=mus[:, i : i + 1], in0=mus[:, i : i + 1],
                            scalar1=1.0 / D,
                        )
                        # var = (sqsa+sqs)/D - mu^2
                        if dc > 0 and dc < D:
                            nc.vector.tensor_add(
                                out=sqs[:, i : i + 1], in0=sqs[:, i : i + 1],
                                in1=sqsa[:, i : i + 1],
                            )
                        elif dc == D:
                            nc.vector.tensor_copy(
                                out=sqs[:, i : i + 1], in_=sqsa[:, i : i + 1]
                            )
                        mu2 = spool.tile([128, 1], F32, name="mu2", bufs=4)
                        nc.vector.tensor_mul(
                            out=mu2, in0=mus[:, i : i + 1], in1=mus[:, i : i + 1]
                        )
                        nc.vector.scalar_tensor_tensor(
                            out=gvar[:, i : i + 1],
                            in0=sqs[:, i : i + 1],
                            scalar=1.0 / D,
                            in1=mu2,
                            op0=mybir.AluOpType.mult,
                            op1=mybir.AluOpType.subtract,
                        )

                def emit_sqrt(fg):
                    """one sqrt for the whole group (single act-table swap)"""
                    sums, sqsa, sqs, mus, gvar, outs = gstate[fg]
                    git = len(outs)
                    srt = spool.tile([128, GIT], F32, name="srt", bufs=2)
                    gstate[fg] = (sums, sqsa, sqs, mus, gvar, outs, srt)
                    with tc.high_priority():
                        nc.scalar.activation(
                            out=srt[:, :git], in_=gvar[:, :git],
                            func=mybir.ActivationFunctionType.Sqrt,
                            bias=eps_sb,
                        )

                def emit_apply_store(fg, which):
                    sums, sqsa, sqs, mus, gvar, outs, srt = gstate[fg]
                    g = fg % NG
                    gpos, gt = groups[g]
                    tsl = slice(gpos, gpos + gt)
                    for i in which:
                        with tc.high_priority():
                            rs = spool.tile([128, 1], F32, name="rs", bufs=4)
                            nc.vector.reciprocal(out=rs, in_=srt[:, i : i + 1])
                        nc.vector.tensor_scalar(
                            out=outs[i],
                            in0=outs[i],
                            scalar1=mus[:, i : i + 1],
                            scalar2=rs,
                            op0=mybir.AluOpType.subtract,
                            op1=mybir.AluOpType.mult,
                        )
                        if use_lng:
                            nc.vector.tensor_mul(out=outs[i], in0=outs[i], in1=lng_sb)
                        if use_lnb:
                            nc.vector.tensor_add(out=outs[i], in0=outs[i], in1=lnb_sb)
                        store_eng = {"g": nc.gpsimd, "s": nc.scalar, "y": nc.sync}[
                            store_rings[i % len(store_rings)]
                        ]
                        store_eng.dma_start(
                            out=out_e[tsl, :].rearrange("(i p) d -> p i d", p=128)[
                                :, i, :
                            ],
                            in_=outs[i],
                        )

                def emit_L2(idx):
                    g, p = blocks[idx]
                    fg = idx // NP
                    n0 = 2 * p
                    gt = groups[g][1]
                    git = gt // 128
                    if p == 0:
                        sums = spool.tile([128, GIT, NP], F32, name="sums")
                        sqsa = spool.tile([128, GIT], F32, name="sqsa")
                        sqs = spool.tile([128, GIT], F32, name="sqs")
                        mus = spool.tile([128, GIT], F32, name="mus")
                        gvar = spool.tile([128, GIT], F32, name="gvar")
                        outs = [
                            opool.tile([128, D], BF16, name=f"o{i}")
                            for i in range(git)
                        ]
                        gstate[fg] = (sums, sqsa, sqs, mus, gvar, outs)
                    sums, sqsa, sqs, mus, gvar, outs = gstate[fg][:6]
                    hT = hT_tiles.pop(idx)
                    xr_sb = xr_tiles.pop(idx)
                    for i in range(git):
                        py = pp_y.tile([128, 2 * CH], F32, tag="py", name="py")
                        for nn in range(2):
                            n = n0 + nn
                            if FP8:
                                for a in range(KT // 2):
                                    nc.tensor.matmul(
                                        py[:, nn * CH : (nn + 1) * CH],
                                        hT[:, nn, 2 * a : 2 * a + 2,
                                           i * 128 : (i + 1) * 128],
                                        w2_sb[n][:, 2 * a : 2 * a + 2, :],
                                        start=(a == 0),
                                        stop=(a == KT // 2 - 1),
                                        perf_mode=DR,
                                    )
                            else:
                                for j in range(KT):
                                    nc.tensor.matmul(
                                        py[:, nn * CH : (nn + 1) * CH],
                                        hT[:, nn, j, i * 128 : (i + 1) * 128],
                                        w2_sb[n][:, j, :],
                                        start=(j == 0),
                                        stop=(j == KT - 1),
                                    )
                        osl = outs[i][:, n0 * CH : (n0 + 2) * CH]
                        # out = y/W2_SCALE + x ; accumulate per-token pair sums
                        nc.vector.scalar_tensor_tensor(
                            out=osl,
                            in0=py,
                            scalar=(1.0 / W2_SCALE) if FP8 else 1.0,
                            in1=xr_sb[:, i, :],
                            op0=mybir.AluOpType.mult,
                            op1=mybir.AluOpType.add,
                            accum_out=sums[:, i, p : p + 1],
                        )
                        if use_b2:
                            for nn in range(2):
                                nc.vector.tensor_add(
                                    outs[i][:, (n0 + nn) * CH : (n0 + nn + 1) * CH],
                                    outs[i][:, (n0 + nn) * CH : (n0 + nn + 1) * CH],
                                    b2_sb[:, n0 + nn, :],
                                )

                def emit_deferred(idx):
                    """LN tail work for earlier groups, spread one piece per
                    block: squares+stats for group fg-1 (tile p); at p==3
                    sqrt + applies {0,1}; at p==0 applies {2,3} of fg-2."""
                    fg, p = idx // NP, idx % NP
                    if p == 0 and fg >= 2 and (fg - 2) in gstate:
                        emit_apply_store(fg - 2, range(2, len(gstate[fg - 2][5])))
                        gstate.pop(fg - 2)
                    if fg >= 1 and (fg - 1) in gstate:
                        git = len(gstate[fg - 1][5])
                        if p < git:
                            emit_sq_stats(fg - 1, p)
                        if p == NP - 1:
                            emit_sqrt(fg - 1)
                            emit_apply_store(fg - 1, range(0, 2))

                for idx in range(min(pf, nblocks)):
                    emit_xdma(idx)
                for idx in range(nblocks):
                    g, p = blocks[idx]
                    if idx < NP:
                        n0 = 2 * p
                        load_w(1, n0 + 4)
                        load_w(1, n0 + 5)
                        load_w(2, n0 + 2)
                        load_w(2, n0 + 3)
                    if idx + pf < nblocks:
                        emit_xdma(idx + pf)
                    emit_L1(idx)
                    if idx >= 1:
                        emit_L2(idx - 1)
                    emit_deferred(idx)
                emit_L2(nblocks - 1)
                # epilogue: flush deferred LN tail for the final groups
                for idx in range(nblocks, nblocks + 2 * NP):
                    emit_deferred(idx)
                assert not gstate, (list(gstate),)

    _split_excess_waits(nc)
    return nc


_CACHE = {}


def prep_inputs(inputs):
    """Host-side sharding + layout prep -> per-core input maps."""
    x = np.asarray(inputs["x"])
    w1 = np.asarray(inputs["w1"], dtype=np.float32)
    w2 = np.asarray(inputs["w2"], dtype=np.float32)
    b1 = np.asarray(inputs["b1"], dtype=np.float32)
    b2 = np.asarray(inputs["b2"], dtype=np.float32)
    ln_g = np.asarray(inputs["ln_g"], dtype=np.float32)
    ln_b = np.asarray(inputs["ln_b"], dtype=np.float32)
    B, L, d = x.shape
    x2 = np.ascontiguousarray(x.reshape(B * L, D).astype(np.float32))
    _w1p = w1.reshape(NCH, KT, 128, CH).transpose(0, 2, 1, 3)
    w1h = np.ascontiguousarray(
        (_w1p * W1_SCALE).astype(NPF8) if FP8_L1 else _w1p.astype(BF)
    )
    _w2p = w2.reshape(NCH, KT, 128, CH).transpose(0, 2, 1, 3)
    w2h = np.ascontiguousarray(
        (_w2p * W2_SCALE).astype(NPF8) if FP8 else _w2p.astype(BF)
    )
    b1h = np.ascontiguousarray(
        b1.reshape(NCH, KT, 128).transpose(2, 0, 1).reshape(128, NCH * KT)
    )

    in_maps = []
    for c in range(N_CORES):
        rows = x2[c * S : (c + 1) * S]  # [S, D] fp32
        xTh = (
            np.ascontiguousarray(rows.T)
            .astype(NPF8 if FP8_L1 else BF)
            .reshape(NCH, KT, 128, S)
        )
        in_maps.append(
            {
                "xT": xTh,
                "xr": rows.astype(BF),
                "w1": w1h,
                "w2": w2h,
                "b1c": b1h,
                "b2": b2,
                "ln_g": ln_g,
                "ln_b": ln_b,
            }
        )
    return in_maps


def kernel(x, w1, b1, w2, b2, ln_g, ln_b):
    x = np.asarray(x)
    b1 = np.asarray(b1, dtype=np.float32)
    b2 = np.asarray(b2, dtype=np.float32)
    ln_g = np.asarray(ln_g, dtype=np.float32)
    ln_b = np.asarray(ln_b, dtype=np.float32)
    B, L, d = x.shape
    assert d == D and B * L == N_CORES * S, (x.shape,)

    use_b1 = bool(np.any(b1 != 0.0))
    use_b2 = bool(np.any(b2 != 0.0))
    use_lng = bool(np.any(ln_g != 1.0))
    use_lnb = bool(np.any(ln_b != 0.0))

    key = (use_b2, use_lng, use_lnb, use_b1)
    if key not in _CACHE:
        _CACHE[key] = _build(*key[:3], use_b1=use_b1)
    nc = _CACHE[key]

    in_maps = prep_inputs(
        {"x": x, "w1": w1, "b1": b1, "w2": w2, "b2": b2, "ln_g": ln_g, "ln_b": ln_b}
    )

    res = run_bass_kernel_spmd(nc, in_maps, list(range(N_CORES)))
    out = np.concatenate([res.results[c]["out"] for c in range(N_CORES)], axis=0)
    return out.reshape(B, L, D).astype(np.float32)
